# revision 1
# baseline (speedup 1.0000x reference)
"""GQA attention kernel for Trainium2 (8 NeuronCores).

Problem: B=2, S=2048, D=2048, H=16 heads of DH=128, KV=4 kv heads, G=4
query heads per kv head.  Full (dense) attention, fp32 I/O.

Sharding: batch (2) x kv-head (4) = 8 cores, zero redundant FLOPs.
Each core computes, for its (batch b, kv head h):
    Q_g = x_b @ Wq[:, h,g]  (4 query heads), K = x_b @ Wk[:, h],
    V = x_b @ Wv[:, h], O_g = softmax(Q_g K^T / sqrt(DH)) V,
    y_partial = concat_g(O_g) @ Wo[h-rows, :]
Host sums the 4 kv-head partials per batch and adds bo.

On-chip strategy (all matmuls bf16 with fp32 PSUM accumulation):
 - host pre-transposes x (xT: [D, S]) and pre-casts weights to bf16
 - QT/KT computed head-transposed ([dh, s]) with W stationary, xT moving
 - S^T tiles ([k, q]) computed directly (KT-slice stationary, QT moving)
   so exp(S^T) lands in SBUF already transposed for the AV matmul:
   no P-transpose pass, no max-subtraction (scores are O(few), exp safe)
 - rowsum via ones-vector matmul accumulated alongside AV
 - softmax normalization folded into the PSUM->SBUF copy of O^T
   (multiply by DMA-broadcast 1/rowsum row)
 - out-proj: O^T stationary, Wo moving -> y natural, DMA'd straight
   from PSUM to DRAM.
"""

import sys

if "/opt/trn_rl_repo" not in sys.path:
    sys.path.insert(0, "/opt/trn_rl_repo")

import numpy as np
import ml_dtypes
from contextlib import ExitStack

B, S, D = 2, 2048, 2048
H, DH, GRP = 16, 128, 4
KV = H // GRP            # 4 kv heads
EH = GRP * DH            # 512 = query-head columns per kv head
SCALE = float(1.0 / np.sqrt(np.float32(DH)))
P = 128                  # partitions
NB = 512                 # matmul moving-dim block (one PSUM bank fp32)


def _emit(ctx, tc, aps, s=S, d=D, debug_taps=None):
    """Emit the per-core program. s, d parameterized for small-shape sim tests."""
    import concourse.bass as bass
    from concourse import mybir

    nc = tc.nc
    bf16 = mybir.dt.bfloat16
    f32 = mybir.dt.float32
    Exp = mybir.ActivationFunctionType.Exp
    Identity = mybir.ActivationFunctionType.Identity

    xt, wq, wk, wv, wo, bq, bk, bv, y = (
        aps["xt"], aps["wq"], aps["wk"], aps["wv"], aps["wo"],
        aps["bq"], aps["bk"], aps["bv"], aps["y"],
    )
    nt = s // P           # number of 128-tiles along s
    nd = d // P           # number of 128-tiles along d (contraction)
    nsb = s // NB         # number of 512-blocks along s
    ndb = d // NB         # number of 512-blocks along d (out columns)

    persist = ctx.enter_context(tc.tile_pool(name="persist", bufs=1))
    psum = ctx.enter_context(tc.tile_pool(name="psum", bufs=2, space="PSUM"))
    ptpool = ctx.enter_context(tc.tile_pool(name="ptp", bufs=2))
    rpool = ctx.enter_context(tc.tile_pool(name="rp", bufs=2))
    projpool = tc.tile_pool(name="projp", bufs=1)
    projp = projpool.__enter__()

    xt_sb = projp.tile([P, nd, s], bf16)
    wq_sb = projp.tile([P, nd, EH], bf16)
    wk_sb = projp.tile([P, nd, DH], bf16)
    wv_sb = projp.tile([P, nd, DH], bf16)
    wo_sb = persist.tile([P, GRP, d], bf16)
    qt_sb = persist.tile([P, GRP, s], bf16)
    kt_sb = persist.tile([P, s], bf16)
    v_sb = persist.tile([P, nt, DH], bf16)
    ot_sb = persist.tile([P, GRP, s], bf16)
    bq_sb = persist.tile([P, GRP], f32)
    bk_sb = persist.tile([P, 1], f32)
    bvb_sb = persist.tile([P, DH], f32)
    ones_sb = persist.tile([P, 1], bf16)

    nc.vector.memset(ones_sb, 1.0)

    # ---- loads ----
    xt_r = xt.rearrange("(t p) s -> p t s", p=P)
    wq_r = wq.rearrange("(t p) e -> p t e", p=P)
    wk_r = wk.rearrange("(t p) e -> p t e", p=P)
    wv_r = wv.rearrange("(t p) e -> p t e", p=P)
    wo_r = wo.rearrange("(g p) d -> p g d", p=P)
    for t in range(nd):
        nc.sync.dma_start(out=xt_sb[:, t, :], in_=xt_r[:, t, :])
        nc.sync.dma_start(out=wq_sb[:, t, :], in_=wq_r[:, t, :])
        nc.sync.dma_start(out=wk_sb[:, t, :], in_=wk_r[:, t, :])
        nc.sync.dma_start(out=wv_sb[:, t, :], in_=wv_r[:, t, :])
    for g in range(GRP):
        nc.sync.dma_start(out=wo_sb[:, g, :], in_=wo_r[:, g, :])
    nc.sync.dma_start(out=bq_sb, in_=bq.rearrange("(g p) -> p g", p=P))
    nc.sync.dma_start(out=bk_sb, in_=bk.rearrange("(p o) -> p o", o=1))
    # bv broadcast across partitions (varies along free dim of V)
    bv_bcast = bass.AP(tensor=bv.tensor, offset=bv.offset,
                       ap=[[0, P]] + list(bv.ap))
    nc.sync.dma_start(out=bvb_sb, in_=bv_bcast)

    # ---- projections ----
    # QT_g [dh, s] = (Wq_g)^T x^T, + bq*scale, scaled by 1/sqrt(DH)
    for g in range(GRP):
        for sb in range(nsb):
            ps = psum.tile([P, NB], f32, tag="mm")
            for t in range(nd):
                nc.tensor.matmul(
                    ps,
                    lhsT=wq_sb[:, t, g * DH:(g + 1) * DH],
                    rhs=xt_sb[:, t, sb * NB:(sb + 1) * NB],
                    start=(t == 0), stop=(t == nd - 1),
                )
            nc.scalar.activation(
                out=qt_sb[:, g, sb * NB:(sb + 1) * NB], in_=ps,
                func=Identity, bias=bq_sb[:, g:g + 1], scale=SCALE,
            )
    # KT [dh, s]
    for sb in range(nsb):
        ps = psum.tile([P, NB], f32, tag="mm")
        for t in range(nd):
            nc.tensor.matmul(
                ps, lhsT=wk_sb[:, t, :], rhs=xt_sb[:, t, sb * NB:(sb + 1) * NB],
                start=(t == 0), stop=(t == nd - 1),
            )
        nc.scalar.activation(
            out=kt_sb[:, sb * NB:(sb + 1) * NB], in_=ps,
            func=Identity, bias=bk_sb[:, 0:1], scale=1.0,
        )
    # V natural [k, dh] (xT stationary)
    for ki in range(nt):
        ps = psum.tile([P, NB], f32, tag="mm")
        for t in range(nd):
            nc.tensor.matmul(
                ps[:, 0:DH], lhsT=xt_sb[:, t, ki * P:(ki + 1) * P],
                rhs=wv_sb[:, t, :],
                start=(t == 0), stop=(t == nd - 1),
            )
        nc.vector.tensor_add(v_sb[:, ki, :], ps[:, 0:DH], bvb_sb)

    projpool.__exit__(None, None, None)

    # ---- attention ----
    # Software-pipelined: block n's S^T/exp interleave with block n-1's
    # AV + rowsum matmuls so PE never stalls waiting for ScalarE's exp.
    blocks = [(g, qb) for g in range(GRP) for qb in range(nsb)]

    def finish_block(prev):
        pg, pqb, ppt, pps_o, pps_r = prev
        pqsl = slice(pqb * NB, (pqb + 1) * NB)
        rrow = rpool.tile([1, NB], f32, tag="rrow")
        nc.vector.reciprocal(rrow, pps_r)
        rb = rpool.tile([P, NB], f32, tag="rb")
        nc.gpsimd.partition_broadcast(rb, rrow[0:1, :])
        nc.vector.tensor_mul(ot_sb[:, pg, pqsl], pps_o, rb)

    prev = None
    for g, qb in blocks:
        qsl = slice(qb * NB, (qb + 1) * NB)
        pt = ptpool.tile([P, nt, NB], bf16, tag="pt")
        ps_o = psum.tile([P, NB], f32, tag="o")
        ps_r = psum.tile([1, NB], f32, tag="r")
        for ki in range(nt):
            ps_s = psum.tile([P, NB], f32, tag="s")
            nc.tensor.matmul(
                ps_s, lhsT=kt_sb[:, ki * P:(ki + 1) * P],
                rhs=qt_sb[:, g, qsl], start=True, stop=True,
            )
            nc.scalar.activation(out=pt[:, ki, :], in_=ps_s, func=Exp)
            if prev is not None:
                _, _, ppt, pps_o, pps_r = prev
                nc.tensor.matmul(
                    pps_o, lhsT=v_sb[:, ki, :], rhs=ppt[:, ki, :],
                    start=(ki == 0), stop=(ki == nt - 1),
                )
                nc.tensor.matmul(
                    pps_r, lhsT=ones_sb[:, 0:1], rhs=ppt[:, ki, :],
                    start=(ki == 0), stop=(ki == nt - 1),
                )
        if prev is not None:
            finish_block(prev)
        prev = (g, qb, pt, ps_o, ps_r)
    # drain last block
    g, qb, pt, ps_o, ps_r = prev
    for ki in range(nt):
        nc.tensor.matmul(
            ps_o, lhsT=v_sb[:, ki, :], rhs=pt[:, ki, :],
            start=(ki == 0), stop=(ki == nt - 1),
        )
        nc.tensor.matmul(
            ps_r, lhsT=ones_sb[:, 0:1], rhs=pt[:, ki, :],
            start=(ki == 0), stop=(ki == nt - 1),
        )
    finish_block(prev)

    if debug_taps is not None:
        for name, t in [("qt", qt_sb), ("kt", kt_sb), ("v", v_sb),
                        ("ot", ot_sb), ("pt_last", None)]:
            if name in debug_taps and t is not None:
                nc.sync.dma_start(out=debug_taps[name], in_=t[:])

    # ---- out projection ----
    ypool = ctx.enter_context(tc.tile_pool(name="yp", bufs=2))
    for st in range(nt):
        for db in range(ndb):
            ps_y = psum.tile([P, NB], f32, tag="mm")
            for g in range(GRP):
                nc.tensor.matmul(
                    ps_y, lhsT=ot_sb[:, g, st * P:(st + 1) * P],
                    rhs=wo_sb[:, g, db * NB:(db + 1) * NB],
                    start=(g == 0), stop=(g == GRP - 1),
                )
            y_sb = ypool.tile([P, NB], f32, tag="y")
            if (st * ndb + db) % 2 == 0:
                nc.scalar.copy(y_sb, ps_y)
            else:
                nc.vector.tensor_copy(y_sb, ps_y)
            nc.sync.dma_start(
                out=y[st * P:(st + 1) * P, db * NB:(db + 1) * NB], in_=y_sb)


def build_program(s=S, d=D, debug=False):
    import concourse.tile as tile
    from concourse import bacc, mybir

    nc = bacc.Bacc("TRN2", target_bir_lowering=False, debug=False)
    bf16 = mybir.dt.bfloat16
    f32 = mybir.dt.float32
    aps = {
        "xt": nc.dram_tensor("xt", [d, s], bf16, kind="ExternalInput").ap(),
        "wq": nc.dram_tensor("wq", [d, EH], bf16, kind="ExternalInput").ap(),
        "wk": nc.dram_tensor("wk", [d, DH], bf16, kind="ExternalInput").ap(),
        "wv": nc.dram_tensor("wv", [d, DH], bf16, kind="ExternalInput").ap(),
        "wo": nc.dram_tensor("wo", [EH, d], bf16, kind="ExternalInput").ap(),
        "bq": nc.dram_tensor("bq", [EH], f32, kind="ExternalInput").ap(),
        "bk": nc.dram_tensor("bk", [DH], f32, kind="ExternalInput").ap(),
        "bv": nc.dram_tensor("bv", [DH], f32, kind="ExternalInput").ap(),
        "y": nc.dram_tensor("y", [s, d], f32, kind="ExternalOutput").ap(),
    }
    debug_taps = None
    if debug:
        nt = s // P
        debug_taps = {
            "qt": nc.dram_tensor("dbg_qt", [P, GRP, s], bf16, kind="ExternalOutput").ap(),
            "kt": nc.dram_tensor("dbg_kt", [P, s], bf16, kind="ExternalOutput").ap(),
            "v": nc.dram_tensor("dbg_v", [P, nt, DH], bf16, kind="ExternalOutput").ap(),
            "ot": nc.dram_tensor("dbg_ot", [P, GRP, s], bf16, kind="ExternalOutput").ap(),
        }
    with tile.TileContext(nc) as tc:
        with ExitStack() as ctx:
            _emit(ctx, tc, aps, s=s, d=d, debug_taps=debug_taps)
    nc.compile()
    return nc


def make_in_maps(x, Wq, bq, Wk, bk, Wv, bv, Wo, bo):
    bf = ml_dtypes.bfloat16
    in_maps = []
    for b in range(B):
        xt_b = x[b].T.astype(bf)  # [D, S] contiguous
        for h in range(KV):
            in_maps.append({
                "xt": xt_b,
                "wq": Wq[:, h * EH:(h + 1) * EH].astype(bf),
                "wk": Wk[:, h * DH:(h + 1) * DH].astype(bf),
                "wv": Wv[:, h * DH:(h + 1) * DH].astype(bf),
                "wo": np.ascontiguousarray(Wo[h * EH:(h + 1) * EH, :]).astype(bf),
                "bq": (bq[h * EH:(h + 1) * EH] * SCALE).astype(np.float32),
                "bk": np.ascontiguousarray(bk[h * DH:(h + 1) * DH]).astype(np.float32),
                "bv": np.ascontiguousarray(bv[h * DH:(h + 1) * DH]).astype(np.float32),
            })
    return in_maps


_PROG = None


def _get_program():
    global _PROG
    if _PROG is None:
        _PROG = build_program()
    return _PROG


def run_cores(in_maps, trace=False, **kw):
    from concourse.bass_utils import run_bass_kernel_spmd
    nc = _get_program()
    return run_bass_kernel_spmd(nc, in_maps, list(range(8)), trace=trace, **kw)


def kernel(**inputs):
    x = np.asarray(inputs["x"], dtype=np.float32)
    Wq = np.asarray(inputs["Wq"], dtype=np.float32)
    bq = np.asarray(inputs["bq"], dtype=np.float32)
    Wk = np.asarray(inputs["Wk"], dtype=np.float32)
    bk = np.asarray(inputs["bk"], dtype=np.float32)
    Wv = np.asarray(inputs["Wv"], dtype=np.float32)
    bv = np.asarray(inputs["bv"], dtype=np.float32)
    Wo = np.asarray(inputs["Wo"], dtype=np.float32)
    bo = np.asarray(inputs["bo"], dtype=np.float32)

    in_maps = make_in_maps(x, Wq, bq, Wk, bk, Wv, bv, Wo, bo)
    res = run_cores(in_maps)
    out = np.empty((B, S, D), dtype=np.float32)
    for b in range(B):
        acc = res.results[b * KV]["y"].astype(np.float32)
        for h in range(1, KV):
            acc = acc + res.results[b * KV + h]["y"]
        out[b] = acc + bo[None, :]
    return out



# revision 10
# speedup vs baseline: 1.4894x; 1.4894x over previous
"""GQA attention kernel for Trainium2 (8 NeuronCores).

Problem: B=2, S=2048, D=2048, H=16 heads of DH=128, KV=4 kv heads, G=4
query heads per kv head.  Full (dense) attention, fp32 I/O.

Sharding: batch (2) x kv-head (4) = 8 cores, zero redundant FLOPs.
Each core computes, for its (batch b, kv head h):
    Q_g = x_b @ Wq[:, h,g]  (4 query heads), K = x_b @ Wk[:, h],
    V = x_b @ Wv[:, h], O_g = softmax(Q_g K^T / sqrt(DH)) V,
    y_partial = concat_g(O_g) @ Wo[h-rows, :]   (bf16)
Host sums the 4 kv-head partials per batch and adds bo.

On-chip strategy (all matmuls bf16 with fp32 PSUM accumulation):
 - host pre-transposes x (xT: [D, S]) and pre-casts weights to bf16
 - projections run as wave-interleaved accumulation chains (all chains
   step together over the contraction tiles) so compute overlaps the
   input DMA stream instead of waiting for the full xT load
 - S^T tiles ([k, q]) computed directly (KT-slice stationary, QT moving)
   so exp(S^T) lands in SBUF already transposed for the AV matmul
 - softmax rowsum via 4 tiny matmuls per k-tile: exp-tile subblock as
   stationary, a ones column as moving -> out [q,1], ~1 PE row each
   (vs. streaming all of exp(S^T) again with a ones-row stationary)
 - 1/rowsum transposed back to a row with one [128,4] PE transpose,
   partition-broadcast on GpSimd, normalization fused into the
   PSUM->SBUF copy of O^T on DVE
 - out-proj matmuls for query-block qb are interleaved one-per-k-tile
   into the attention inner loop of later blocks, filling the PE gap
   left while ScalarE computes exp (PE stays busy; exp is off the
   critical path)
 - y stored bf16, one DMA per 128-row stripe
"""

import sys

if "/opt/trn_rl_repo" not in sys.path:
    sys.path.insert(0, "/opt/trn_rl_repo")

import numpy as np
import ml_dtypes
from contextlib import ExitStack

B, S, D = 2, 2048, 2048
H, DH, GRP = 16, 128, 4
KV = H // GRP            # 4 kv heads
EH = GRP * DH            # 512 = query-head columns per kv head
SCALE = float(1.0 / np.sqrt(np.float32(DH)))
P = 128                  # partitions
NB = 512                 # matmul moving-dim block (one PSUM bank fp32)


def _emit(ctx, tc, aps, s=S, d=D):
    """Emit the per-core program. s, d parameterized for small-shape sim tests."""
    import concourse.bass as bass
    from concourse import mybir

    nc = tc.nc
    bf16 = mybir.dt.bfloat16
    f32 = mybir.dt.float32
    Exp = mybir.ActivationFunctionType.Exp
    Identity = mybir.ActivationFunctionType.Identity

    xt, wq, wk, wv, wo, bq, bk, bv, idt, y = (
        aps["xt"], aps["wq"], aps["wk"], aps["wv"], aps["wo"],
        aps["bq"], aps["bk"], aps["bv"], aps["idt"], aps["y"],
    )
    nt = s // P           # number of 128-tiles along s (k tiles)
    nd = d // P           # number of 128-tiles along d (contraction)
    nsb = s // NB         # number of 512-blocks along s
    ndb = d // NB         # number of 512-blocks along d (out columns)

    persist = ctx.enter_context(tc.tile_pool(name="persist", bufs=1))

    wo_sb = persist.tile([P, GRP, d], bf16)
    qt_sb = persist.tile([P, GRP, s], bf16)
    kt_sb = persist.tile([P, s], bf16)
    v_sb = persist.tile([P, nt, DH], bf16)
    ot_sb = persist.tile([P, GRP, s], bf16)
    bq_sb = persist.tile([P, GRP], f32)
    bk_sb = persist.tile([P, 1], f32)
    bvb_sb = persist.tile([P, DH], f32)
    ones_sb = persist.tile([P, 1], bf16)
    idt_sb = persist.tile([P, P], f32)

    nc.vector.memset(ones_sb, 1.0)

    # ---- loads (t-interleaved so projection chains consume tiles as they
    # arrive) ----
    projpool = tc.tile_pool(name="projp", bufs=1)
    projp = projpool.__enter__()
    xt_sb = projp.tile([P, nd, s], bf16)
    wq_sb = projp.tile([P, nd, EH], bf16)
    wk_sb = projp.tile([P, nd, DH], bf16)
    wv_sb = projp.tile([P, nd, DH], bf16)

    xt_r = xt.rearrange("(t p) s -> p t s", p=P)
    wq_r = wq.rearrange("(t p) e -> p t e", p=P)
    wk_r = wk.rearrange("(t p) e -> p t e", p=P)
    wv_r = wv.rearrange("(t p) e -> p t e", p=P)
    wo_r = wo.rearrange("(g p) d -> p g d", p=P)
    for t in range(nd):
        nc.sync.dma_start(out=xt_sb[:, t, :], in_=xt_r[:, t, :])
        nc.sync.dma_start(out=wk_sb[:, t, :], in_=wk_r[:, t, :])
        nc.sync.dma_start(out=wv_sb[:, t, :], in_=wv_r[:, t, :])
    nc.sync.dma_start(out=bk_sb, in_=bk.rearrange("(p o) -> p o", o=1))
    # bv broadcast across partitions (varies along free dim of V)
    bv_bcast = bass.AP(tensor=bv.tensor, offset=bv.offset,
                       ap=[[0, P]] + list(bv.ap))
    nc.sync.dma_start(out=bvb_sb, in_=bv_bcast)
    for t in range(nd):
        nc.sync.dma_start(out=wq_sb[:, t, :], in_=wq_r[:, t, :])
    nc.sync.dma_start(out=bq_sb, in_=bq.rearrange("(g p) -> p g", p=P))
    for g in range(GRP):
        nc.sync.dma_start(out=wo_sb[:, g, :], in_=wo_r[:, g, :])
    nc.sync.dma_start(out=idt_sb, in_=idt)

    # ---- projections (wave-interleaved chains; 8 PSUM banks per wave) ----
    with tc.tile_pool(name="pjps", bufs=1, space="PSUM") as pj:
        # wave 1: KT (4 chains) + V natural (16 chains packed 4/bank)
        kps = [pj.tile([P, NB], f32, tag=f"p{i}", name=f"kps{i}")
               for i in range(4)]
        vps = [pj.tile([P, 4, DH], f32, tag=f"p{4 + i}", name=f"vps{i}")
               for i in range(4)]
        for t in range(nd):
            for sb in range(nsb):
                nc.tensor.matmul(
                    kps[sb], lhsT=wk_sb[:, t, :],
                    rhs=xt_sb[:, t, sb * NB:(sb + 1) * NB],
                    start=(t == 0), stop=(t == nd - 1),
                )
            for ki in range(nt):
                # 4 chains packed per PSUM bank: one accumulation group per
                # bank (start zeroes the whole 2KB zero-region lazily, so
                # only the first write may set start, only the last stop)
                nc.tensor.matmul(
                    vps[ki // 4][:, ki % 4, :],
                    lhsT=xt_sb[:, t, ki * P:(ki + 1) * P],
                    rhs=wv_sb[:, t, :],
                    start=(t == 0 and ki % 4 == 0),
                    stop=(t == nd - 1 and ki % 4 == 3),
                )
        for sb in range(nsb):
            nc.scalar.activation(
                out=kt_sb[:, sb * NB:(sb + 1) * NB], in_=kps[sb],
                func=Identity, bias=bk_sb[:, 0:1], scale=1.0,
            )
        for ki in range(nt):
            nc.vector.tensor_add(v_sb[:, ki, :], vps[ki // 4][:, ki % 4, :],
                                 bvb_sb)
        # waves 2/3: QT, two head-pairs at a time (8 chains each)
        for g0 in range(0, GRP, 2):
            qps = [pj.tile([P, NB], f32, tag=f"p{i}", name=f"qps{i}")
                   for i in range(8)]
            chains = [(g0 + gg, sb) for gg in range(2) for sb in range(nsb)]
            for t in range(nd):
                for i, (g, sb) in enumerate(chains):
                    nc.tensor.matmul(
                        qps[i], lhsT=wq_sb[:, t, g * DH:(g + 1) * DH],
                        rhs=xt_sb[:, t, sb * NB:(sb + 1) * NB],
                        start=(t == 0), stop=(t == nd - 1),
                    )
            for i, (g, sb) in enumerate(chains):
                nc.scalar.activation(
                    out=qt_sb[:, g, sb * NB:(sb + 1) * NB], in_=qps[i],
                    func=Identity, bias=bq_sb[:, g:g + 1], scale=SCALE,
                )
    projpool.__exit__(None, None, None)

    # ---- attention + interleaved out-projection ----
    spool = ctx.enter_context(tc.tile_pool(name="sps", bufs=2, space="PSUM"))
    opool = ctx.enter_context(tc.tile_pool(name="ops", bufs=2, space="PSUM"))
    rpool = ctx.enter_context(tc.tile_pool(name="rps", bufs=1, space="PSUM"))
    rtpool = ctx.enter_context(tc.tile_pool(name="rtps", bufs=1, space="PSUM"))
    ypool = ctx.enter_context(tc.tile_pool(name="yps", bufs=2, space="PSUM"))
    ptpool = ctx.enter_context(tc.tile_pool(name="ptp", bufs=2))
    rsbpool = ctx.enter_context(tc.tile_pool(name="rsb", bufs=2))
    rbpool = ctx.enter_context(tc.tile_pool(name="rbp", bufs=2))
    ysbpool = ctx.enter_context(tc.tile_pool(name="ysb", bufs=2))

    y_st = y.rearrange("(t p) (a b) -> t p a b", p=P, a=ndb)

    # out-projection work queue: one PE matmul per item, interleaved into
    # the attention k-tile loop.  Items for query-block qb become available
    # once all 4 heads of qb are normalized into ot_sb.
    yps_box = {}
    ysb_box = {}

    def emit_outproj_item(it):
        st, db, g = it
        if g == 0:
            yps_box[(st, db)] = ypool.tile([P, NB], f32, tag="y", name="psy")
        ps_y = yps_box[(st, db)]
        nc.tensor.matmul(
            ps_y, lhsT=ot_sb[:, g, st * P:(st + 1) * P],
            rhs=wo_sb[:, g, db * NB:(db + 1) * NB],
            start=(g == 0), stop=(g == GRP - 1),
        )
        if g == GRP - 1:
            if db == 0:
                ysb_box[st] = ysbpool.tile([P, ndb, NB], bf16, tag="ysb",
                                           name="ysb")
            nc.vector.tensor_copy(ysb_box[st][:, db, :],
                                  yps_box.pop((st, db)))
            if db == ndb - 1:
                nc.sync.dma_start(out=y_st[st], in_=ysb_box.pop(st))

    def outproj_gen():
        for qb in range(nsb):
            for st in range(qb * (nt // nsb), (qb + 1) * (nt // nsb)):
                for db in range(ndb):
                    for g in range(GRP):
                        yield (st, db, g)

    opq = outproj_gen()
    avail = 0

    def finish_block(prev):
        pg, pqb, _, pps_o, pps_r = prev
        pqsl = slice(pqb * NB, (pqb + 1) * NB)
        r_sb = rsbpool.tile([P, GRP], f32, tag="rs")
        nc.vector.tensor_copy(r_sb, pps_r)
        ps_rt = rtpool.tile([1, NB], f32, tag="rt")
        for j in range(GRP):
            nc.tensor.transpose(ps_rt[0:1, j * P:(j + 1) * P],
                                r_sb[:, j:j + 1], idt_sb)
        rrow = rsbpool.tile([1, NB], f32, tag="rrow")
        nc.vector.reciprocal(rrow, ps_rt)
        rb = rbpool.tile([P, NB], f32, tag="rb")
        nc.gpsimd.partition_broadcast(rb, rrow[0:1, :])
        nc.vector.tensor_mul(ot_sb[:, pg, pqsl], pps_o, rb)

    blocks = [(g, qb) for qb in range(nsb) for g in range(GRP)]
    prev = None
    for bi, (g, qb) in enumerate(blocks):
        if bi >= 5 and (bi - 5) % GRP == 0:
            avail += (nt // nsb) * ndb * GRP
        qsl = slice(qb * NB, (qb + 1) * NB)
        pt = ptpool.tile([P, nt, NB], bf16, tag="pt")
        ps_o = opool.tile([P, NB], f32, tag="o")
        ps_r = rpool.tile([P, GRP], f32, tag="r")
        for ki in range(nt):
            ps_s = spool.tile([P, NB], f32, tag="s")
            nc.tensor.matmul(
                ps_s, lhsT=kt_sb[:, ki * P:(ki + 1) * P],
                rhs=qt_sb[:, g, qsl], start=True, stop=True,
            )
            nc.scalar.activation(out=pt[:, ki, :], in_=ps_s, func=Exp)
            if prev is not None:
                _, _, ppt, pps_o, pps_r = prev
                nc.tensor.matmul(
                    pps_o, lhsT=v_sb[:, ki, :], rhs=ppt[:, ki, :],
                    start=(ki == 0), stop=(ki == nt - 1),
                )
                for j in range(GRP):
                    # 4 column-chains share one bank: single group
                    nc.tensor.matmul(
                        pps_r[:, j:j + 1],
                        lhsT=ppt[:, ki, j * P:(j + 1) * P], rhs=ones_sb,
                        start=(ki == 0 and j == 0),
                        stop=(ki == nt - 1 and j == GRP - 1),
                    )
            if avail > 0:
                emit_outproj_item(next(opq))
                avail -= 1
        if prev is not None:
            finish_block(prev)
        prev = (g, qb, pt, ps_o, ps_r)
    # drain last block
    _, _, ppt, pps_o, pps_r = prev
    for ki in range(nt):
        nc.tensor.matmul(
            pps_o, lhsT=v_sb[:, ki, :], rhs=ppt[:, ki, :],
            start=(ki == 0), stop=(ki == nt - 1),
        )
        for j in range(GRP):
            nc.tensor.matmul(
                pps_r[:, j:j + 1], lhsT=ppt[:, ki, j * P:(j + 1) * P],
                rhs=ones_sb, start=(ki == 0 and j == 0),
                stop=(ki == nt - 1 and j == GRP - 1),
            )
    finish_block(prev)
    # remaining out-projection work
    for it in opq:
        emit_outproj_item(it)


def build_program(s=S, d=D):
    import concourse.tile as tile
    from concourse import bacc, mybir

    nc = bacc.Bacc("TRN2", target_bir_lowering=False, debug=False)
    bf16 = mybir.dt.bfloat16
    f32 = mybir.dt.float32
    aps = {
        "xt": nc.dram_tensor("xt", [d, s], bf16, kind="ExternalInput").ap(),
        "wq": nc.dram_tensor("wq", [d, EH], bf16, kind="ExternalInput").ap(),
        "wk": nc.dram_tensor("wk", [d, DH], bf16, kind="ExternalInput").ap(),
        "wv": nc.dram_tensor("wv", [d, DH], bf16, kind="ExternalInput").ap(),
        "wo": nc.dram_tensor("wo", [EH, d], bf16, kind="ExternalInput").ap(),
        "bq": nc.dram_tensor("bq", [EH], f32, kind="ExternalInput").ap(),
        "bk": nc.dram_tensor("bk", [DH], f32, kind="ExternalInput").ap(),
        "bv": nc.dram_tensor("bv", [DH], f32, kind="ExternalInput").ap(),
        "idt": nc.dram_tensor("idt", [P, P], f32, kind="ExternalInput").ap(),
        "y": nc.dram_tensor("y", [s, d], bf16, kind="ExternalOutput").ap(),
    }
    with tile.TileContext(nc) as tc:
        with ExitStack() as ctx:
            _emit(ctx, tc, aps, s=s, d=d)
    nc.compile()
    return nc


def make_in_maps(x, Wq, bq, Wk, bk, Wv, bv, Wo, bo):
    bf = ml_dtypes.bfloat16
    idt = np.eye(P, dtype=np.float32)
    in_maps = []
    for b in range(B):
        xt_b = x[b].T.astype(bf)  # [D, S] contiguous
        for h in range(KV):
            in_maps.append({
                "xt": xt_b,
                "wq": Wq[:, h * EH:(h + 1) * EH].astype(bf),
                "wk": Wk[:, h * DH:(h + 1) * DH].astype(bf),
                "wv": Wv[:, h * DH:(h + 1) * DH].astype(bf),
                "wo": np.ascontiguousarray(Wo[h * EH:(h + 1) * EH, :]).astype(bf),
                "bq": (bq[h * EH:(h + 1) * EH] * SCALE).astype(np.float32),
                "bk": np.ascontiguousarray(bk[h * DH:(h + 1) * DH]).astype(np.float32),
                "bv": np.ascontiguousarray(bv[h * DH:(h + 1) * DH]).astype(np.float32),
                "idt": idt,
            })
    return in_maps


_PROG = None


def _get_program():
    global _PROG
    if _PROG is None:
        _PROG = build_program()
    return _PROG


def run_cores(in_maps, trace=False, **kw):
    from concourse.bass_utils import run_bass_kernel_spmd
    nc = _get_program()
    return run_bass_kernel_spmd(nc, in_maps, list(range(8)), trace=trace, **kw)


def kernel(**inputs):
    x = np.asarray(inputs["x"], dtype=np.float32)
    Wq = np.asarray(inputs["Wq"], dtype=np.float32)
    bq = np.asarray(inputs["bq"], dtype=np.float32)
    Wk = np.asarray(inputs["Wk"], dtype=np.float32)
    bk = np.asarray(inputs["bk"], dtype=np.float32)
    Wv = np.asarray(inputs["Wv"], dtype=np.float32)
    bv = np.asarray(inputs["bv"], dtype=np.float32)
    Wo = np.asarray(inputs["Wo"], dtype=np.float32)
    bo = np.asarray(inputs["bo"], dtype=np.float32)

    in_maps = make_in_maps(x, Wq, bq, Wk, bk, Wv, bv, Wo, bo)
    res = run_cores(in_maps)
    out = np.empty((B, S, D), dtype=np.float32)
    for b in range(B):
        acc = res.results[b * KV]["y"].astype(np.float32)
        for h in range(1, KV):
            acc = acc + res.results[b * KV + h]["y"].astype(np.float32)
        out[b] = acc + bo[None, :]
    return out
